# revision 2
# baseline (speedup 1.0000x reference)
"""Trainium2 Bass kernel for CustomAttention (B=4, S=2048, D=1024, H=16).

Sharding: 8 cores = 4 batches x 2 head-halves (head/tensor parallel). Each core
computes Q/K/V projections for its 8 heads (512-dim slice of the projection
weights), attention for those heads over all 2048 queries/keys, and a partial
out-projection (contraction over its 512 columns of Wo). The host sums the two
partial out-projections per batch. No K/V projection duplication, no DRAM
round-trips for intermediates, no collectives.

Precision: fp16 operands for projections and QK^T (0.05% quantization - E
error ~0.006 abs, negligible through softmax), bf16 for the post-exp path
(P, V, attT; bf16 needed for exp's dynamic range, no max-subtraction), fp32
PSUM accumulation everywhere, fp32 final output partials.

Pipeline: per (head-pair, 512-query chunk), a per-key-tile loop streams
QK (PE, tile-position-packed pairs) -> exp (ACT, [128,1024] from PSUM) ->
PV (PE, accumulating) with 2-deep PSUM double-buffering and an 8-deep P-tile
pool; softmax denominator comes from a ones-column appended to V, reciprocal
on DVE, partition-broadcast on GPSIMD. The ACT engine (exp) is the roofline
(~270us); projections and out-projection are emitted interleaved with
attention chunks so their PE work hides under exp.
"""

import math

import numpy as np

B, S, D = 4, 2048, 1024
H, DH = 16, 64
P = 128
HL = 8               # local heads per core
NHP = HL // 2        # 4 head pairs
NKT = S // P         # 16 key tiles
QC = 512             # query chunk
NQC = S // QC        # 4
NDT = D // P         # 8 d_model tiles
SCALE = math.log(D) / math.sqrt(DH)

_CACHE = {}


def _build_nc():
    import concourse.bass as bass
    import concourse.bacc as bacc
    import concourse.mybir as mybir
    import concourse.tile as tile
    from contextlib import ExitStack

    f32 = mybir.dt.float32
    f16 = mybir.dt.float16
    bf16 = mybir.dt.bfloat16
    EXP = mybir.ActivationFunctionType.Exp
    ADD = mybir.AluOpType.add
    MULT = mybir.AluOpType.mult

    nc = bacc.Bacc("TRN2", target_bir_lowering=False, debug=False, num_devices=8)

    xqT = nc.declare_dram_parameter("xqT", [D, S], f16, isOutput=False)
    xkT = nc.declare_dram_parameter("xkT", [D, S], f16, isOutput=False)
    xvT = nc.declare_dram_parameter("xvT", [D, S], f16, isOutput=False)
    WqT = nc.declare_dram_parameter("WqT", [D, 512], f16, isOutput=False)
    WkT = nc.declare_dram_parameter("WkT", [D, 512], f16, isOutput=False)
    WvT = nc.declare_dram_parameter("WvT", [D, 512], f16, isOutput=False)
    WoT = nc.declare_dram_parameter("WoT", [512, D], bf16, isOutput=False)
    bq_d = nc.declare_dram_parameter("bq", [512], f32, isOutput=False)
    bk_d = nc.declare_dram_parameter("bk", [512], f32, isOutput=False)
    bv_d = nc.declare_dram_parameter("bv", [1, 512], f32, isOutput=False)
    bo_d = nc.declare_dram_parameter("bo", [D], f32, isOutput=False)
    outT = nc.declare_dram_parameter("outT", [D, S], f32, isOutput=True)

    with ExitStack() as ctx:
        tc = ctx.enter_context(tile.TileContext(nc))
        persist = ctx.enter_context(tc.tile_pool(name="persist", bufs=1))
        xstage = ctx.enter_context(tc.tile_pool(name="xstage", bufs=3))
        ptp = ctx.enter_context(tc.tile_pool(name="ptp", bufs=8))
        ostage = ctx.enter_context(tc.tile_pool(name="ostage", bufs=2))
        astage = ctx.enter_context(tc.tile_pool(name="astage", bufs=2))
        dnp = ctx.enter_context(tc.tile_pool(name="dnp", bufs=4))
        bcp = ctx.enter_context(tc.tile_pool(name="bcp", bufs=4))
        etp = ctx.enter_context(tc.tile_pool(name="etp", bufs=2, space="PSUM"))
        pvp = ctx.enter_context(tc.tile_pool(name="pvp", bufs=2, space="PSUM"))
        projp = ctx.enter_context(tc.tile_pool(name="projp", bufs=2, space="PSUM"))

        wq = persist.tile([P, NDT, 512], f16, tag="wq")
        wk = persist.tile([P, NDT, 512], f16, tag="wk")
        wv = persist.tile([P, NDT, 512], f16, tag="wv")
        wo = persist.tile([P, 4, D], bf16, tag="wo")
        kT_sb = persist.tile([P, NHP, S], f16, tag="kT")
        qT_sb = persist.tile([P, NHP, S], f16, tag="qT")
        attT = persist.tile([P, NHP, S], bf16, tag="attT")
        v_pad = persist.tile([P, NKT, HL, DH + 1], bf16, tag="v_pad")
        bq_sb = persist.tile([P, 4], f32, tag="bq")
        bk_sb = persist.tile([P, 4], f32, tag="bk")
        bo_sb = persist.tile([P, NDT], f32, tag="bo")
        bv_bc = persist.tile([P, 512], f32, tag="bv_bc")

        # --- setup ---
        nc.sync.dma_start(wq[:], WqT.rearrange("(k p) c -> p k c", p=P))
        nc.sync.dma_start(wk[:], WkT.rearrange("(k p) c -> p k c", p=P))
        nc.sync.dma_start(wv[:], WvT.rearrange("(k p) c -> p k c", p=P))
        nc.sync.dma_start(wo[:], WoT.rearrange("(k p) c -> p k c", p=P))
        nc.sync.dma_start(bq_sb[:], bq_d.rearrange("(o p) -> p o", p=P))
        nc.sync.dma_start(bk_sb[:], bk_d.rearrange("(o p) -> p o", p=P))
        nc.sync.dma_start(bo_sb[:], bo_d.rearrange("(o p) -> p o", p=P))
        nc.sync.dma_start(bv_bc[:], bv_d[:].to_broadcast([P, 512]))
        nc.vector.memset(v_pad[:], 1.0)

        def load_x(x_dram, sc, name):
            xt = xstage.tile([P, NDT, QC], f16, tag="xstage", name=name)
            nc.sync.dma_start(
                xt[:],
                x_dram[:, sc * QC:(sc + 1) * QC].rearrange("(k p) q -> p k q", p=P),
            )
            return xt

        def kq_piece(w_tile, xt, bias_sb, out_sb, hps, sc):
            # projection in [dout, s] form: stationary W, moving x
            for hp in hps:
                ps = projp.tile([P, QC], f32, tag="projp")
                for kt in range(NDT):
                    nc.tensor.matmul(
                        out=ps[:],
                        lhsT=w_tile[:, kt, hp * P:(hp + 1) * P],
                        rhs=xt[:, kt, :],
                        start=(kt == 0), stop=(kt == NDT - 1),
                    )
                nc.vector.tensor_scalar_add(
                    out_sb[:, hp, sc * QC:(sc + 1) * QC], ps[:], bias_sb[:, hp:hp + 1]
                )

        def v_piece(xt, st):
            # projection in [s, dout] form: stationary x, moving W
            so = (st % 4) * P
            ps = projp.tile([P, 512], f32, tag="projp")
            for kt in range(NDT):
                nc.tensor.matmul(
                    out=ps[:],
                    lhsT=xt[:, kt, so:so + P],
                    rhs=wv[:, kt, :],
                    start=(kt == 0), stop=(kt == NDT - 1),
                )
            nc.vector.tensor_tensor(
                v_pad[:, st, :, 0:DH],
                ps[:].rearrange("p (h d) -> p h d", h=HL),
                bv_bc[:].rearrange("p (h d) -> p h d", h=HL),
                ADD,
            )

        def attention_chunk(hp, qc):
            qsl = slice(qc * QC, (qc + 1) * QC)
            pv = [pvp.tile([P, QC], f32, tag="pvp", name=f"pv{hp}_{qc}_{h}")
                  for h in range(2)]
            for kt in range(NKT):
                et = etp.tile([P, 2, QC], f32, tag="etp")
                for h in range(2):
                    nc.tensor.matmul(
                        out=et[:, h, :],
                        lhsT=kT_sb[h * 64:(h + 1) * 64, hp, kt * P:(kt + 1) * P],
                        rhs=qT_sb[h * 64:(h + 1) * 64, hp, qsl],
                        start=True, stop=True,
                        tile_position=(h * 64, 0),
                    )
                pt_t = ptp.tile([P, 2, QC], bf16, tag="ptp")
                nc.scalar.activation(pt_t[:], et[:], EXP, scale=SCALE)
                for h in range(2):
                    nc.tensor.matmul(
                        out=pv[h][0:DH + 1, :],
                        lhsT=v_pad[:, kt, 2 * hp + h, 0:DH + 1],
                        rhs=pt_t[:, h, :],
                        start=(kt == 0), stop=(kt == NKT - 1),
                    )
            # normalize: denominator is PSUM row 64 (ones column of v_pad)
            dn = [dnp.tile([1, QC], f32, tag="dnp", name=f"dn{hp}_{qc}_{h}")
                  for h in range(2)]
            bc = [bcp.tile([64, QC], f32, tag="bcp", name=f"bc{hp}_{qc}_{h}")
                  for h in range(2)]
            for h in range(2):
                nc.vector.reciprocal(dn[h][:], pv[h][DH:DH + 1, :])
                nc.gpsimd.partition_broadcast(bc[h][:], dn[h][:], channels=64)
            nc.vector.tensor_tensor(
                attT[0:64, hp, qsl], pv[0][0:64, :], bc[0][:], MULT
            )
            ast = astage.tile([64, QC], bf16, tag="astage")
            nc.vector.tensor_tensor(ast[:], pv[1][0:64, :], bc[1][:], MULT)
            nc.sync.dma_start(attT[64:128, hp, qsl], ast[:])

        def o_piece(sc):
            ssl = slice(sc * QC, (sc + 1) * QC)
            for dt in range(NDT):
                ps = projp.tile([P, QC], f32, tag="projp")
                for jt in range(4):
                    nc.tensor.matmul(
                        out=ps[:],
                        lhsT=wo[:, jt, dt * P:(dt + 1) * P],
                        rhs=attT[:, jt, ssl],
                        start=(jt == 0), stop=(jt == 3),
                    )
                ot = ostage.tile([P, QC], f32, tag="ostage")
                nc.vector.tensor_scalar_add(ot[:], ps[:], bo_sb[:, dt:dt + 1])
                nc.sync.dma_start(outT[dt * P:(dt + 1) * P, ssl], ot[:])

        # --- schedule ---
        for sc in range(4):
            kq_piece(wk, load_x(xkT, sc, f"xk{sc}"), bk_sb, kT_sb, [0], sc)
        for qc in range(4):
            kq_piece(wq, load_x(xqT, qc, f"xq{qc}"), bq_sb, qT_sb, [0], qc)
        for sc in range(4):
            xvt = load_x(xvT, sc, f"xv{sc}")
            for sti in range(4):
                v_piece(xvt, sc * 4 + sti)
        for qc in range(NQC):
            attention_chunk(0, qc)
            kq_piece(wk, load_x(xkT, qc, f"xk2_{qc}"), bk_sb, kT_sb, [1, 2, 3], qc)
            kq_piece(wq, load_x(xqT, qc, f"xq2_{qc}"), bq_sb, qT_sb, [1, 2, 3], qc)
        for qc in range(NQC):
            attention_chunk(1, qc)
        for qc in range(NQC):
            attention_chunk(2, qc)
        for qc in range(NQC):
            attention_chunk(3, qc)
            o_piece(qc)

    if not nc.is_finalized():
        nc.finalize()
    return nc


def get_nc():
    if "nc" not in _CACHE:
        _CACHE["nc"] = _build_nc()
    return _CACHE["nc"]


def make_in_maps(inputs):
    import ml_dtypes

    bf16 = ml_dtypes.bfloat16
    q = np.asarray(inputs["query"], np.float32)
    k = np.asarray(inputs["key"], np.float32)
    v = np.asarray(inputs["value"], np.float32)
    Wq = np.asarray(inputs["Wq"], np.float32)
    Wk = np.asarray(inputs["Wk"], np.float32)
    Wv = np.asarray(inputs["Wv"], np.float32)
    Wo = np.asarray(inputs["Wo"], np.float32)
    bq = np.asarray(inputs["bq"], np.float32)
    bk = np.asarray(inputs["bk"], np.float32)
    bv = np.asarray(inputs["bv"], np.float32)
    bo = np.asarray(inputs["bo"], np.float32)

    xT = {}
    for b in range(B):
        xT[b] = (
            np.ascontiguousarray(q[b].T).astype(np.float16),
            np.ascontiguousarray(k[b].T).astype(np.float16),
            np.ascontiguousarray(v[b].T).astype(np.float16),
        )
    Wh = {}
    for hh in range(2):
        sl = slice(hh * 512, (hh + 1) * 512)
        Wh[hh] = {
            "WqT": np.ascontiguousarray(Wq.T[:, sl]).astype(np.float16),
            "WkT": np.ascontiguousarray(Wk.T[:, sl]).astype(np.float16),
            "WvT": np.ascontiguousarray(Wv.T[:, sl]).astype(np.float16),
            "WoT": np.ascontiguousarray(Wo[:, sl].T).astype(bf16),
            "bq": np.ascontiguousarray(bq[sl]),
            "bk": np.ascontiguousarray(bk[sl]),
            "bv": np.ascontiguousarray(bv[sl]).reshape(1, 512),
            "bo": (bo * 0.5).astype(np.float32),
        }
    in_maps = []
    for c in range(8):
        b, hh = c // 2, c % 2
        m = dict(Wh[hh])
        m["xqT"], m["xkT"], m["xvT"] = xT[b]
        in_maps.append(m)
    return in_maps


def assemble(results):
    out = np.empty((B, S, D), np.float32)
    for b in range(B):
        acc = results[2 * b]["outT"].astype(np.float32) + \
            results[2 * b + 1]["outT"].astype(np.float32)
        out[b] = acc.T
    return out


def _numpy_fallback(inputs):
    q = np.asarray(inputs["query"], np.float64)
    k = np.asarray(inputs["key"], np.float64)
    v = np.asarray(inputs["value"], np.float64)
    Wq, bq = np.asarray(inputs["Wq"], np.float64), np.asarray(inputs["bq"], np.float64)
    Wk, bk = np.asarray(inputs["Wk"], np.float64), np.asarray(inputs["bk"], np.float64)
    Wv, bv = np.asarray(inputs["Wv"], np.float64), np.asarray(inputs["bv"], np.float64)
    Wo, bo = np.asarray(inputs["Wo"], np.float64), np.asarray(inputs["bo"], np.float64)
    qp = (q @ Wq.T + bq).reshape(B, S, H, DH).transpose(0, 2, 1, 3)
    kp = (k @ Wk.T + bk).reshape(B, S, H, DH).transpose(0, 2, 1, 3)
    vp = (v @ Wv.T + bv).reshape(B, S, H, DH).transpose(0, 2, 1, 3)
    e = np.einsum("bhqd,bhkd->bhqk", qp, kp) * SCALE
    mask = np.asarray(inputs["mask"])
    kpm = np.asarray(inputs["key_padding_mask"])
    e = np.where(mask == 0, -np.inf, e)
    e = np.where(kpm[:, None, None, :] == 0, -np.inf, e)
    e -= e.max(axis=-1, keepdims=True)
    p = np.exp(e)
    p /= p.sum(axis=-1, keepdims=True)
    o = np.einsum("bhqk,bhkd->bhqd", p, vp).transpose(0, 2, 1, 3).reshape(B, S, D)
    return (o @ Wo.T + bo).astype(np.float32)


def kernel(**inputs):
    mask = np.asarray(inputs["mask"])
    kpm = np.asarray(inputs["key_padding_mask"])
    if not (mask.all() and kpm.all()):
        return _numpy_fallback(inputs)
    from concourse.bass_utils import run_bass_kernel_spmd

    nc = get_nc()
    in_maps = make_in_maps(inputs)
    res = run_bass_kernel_spmd(nc, in_maps, list(range(8)))
    return assemble(res.results)


# revision 10
# speedup vs baseline: 1.4096x; 1.4096x over previous
"""Trainium2 Bass kernel for CustomAttention (B=4, S=2048, D=1024, H=16).

Sharding: 8 cores = 4 batches x 2 head-halves (head/tensor parallel). Each core
computes Q/K/V projections for its 8 heads (512-dim slice of the projection
weights), attention for those heads over all 2048 queries/keys, and a partial
out-projection (contraction over its 512 columns of Wo). The host sums the two
partial out-projections per batch. No K/V projection duplication, no DRAM
round-trips for intermediates, no collectives.

Precision: fp16 operands for projections and QK^T (0.05% quantization - E
error ~0.006 abs, negligible through softmax), bf16 for the post-exp path
(P, V, attT; bf16 needed for exp's dynamic range, no max-subtraction), fp32
PSUM accumulation everywhere, fp32 final output partials.

Pipeline: per (head-pair, 512-query chunk), a per-key-tile loop streams
QK (PE, tile-position-packed pairs) -> exp (ACT, [128,1024] from PSUM) ->
PV (PE, accumulating) with 2-deep PSUM double-buffering and an 8-deep P-tile
pool; softmax denominator comes from a ones-column appended to V, reciprocal
on DVE, partition-broadcast on GPSIMD. The ACT engine (exp) is the roofline
(~270us); projections and out-projection are emitted interleaved with
attention chunks so their PE work hides under exp.
"""

import math

import numpy as np

B, S, D = 4, 2048, 1024
H, DH = 16, 64
P = 128
HL = 8               # local heads per core
NHP = HL // 2        # 4 head pairs
NKT = S // P         # 16 key tiles
QC = 512             # query chunk
NQC = S // QC        # 4
NDT = D // P         # 8 d_model tiles
SCALE = math.log(D) / math.sqrt(DH)

import os
KDT = os.environ.get("KDT", "qk32")       # qk32 | f16 | bf16 : Q/K-chain operand dtype
BCAST = os.environ.get("BCAST", "gpsimd")  # gpsimd | pe : denominator broadcast

_CACHE = {}


def _build_nc():
    import concourse.bass as bass
    import concourse.bacc as bacc
    import concourse.mybir as mybir
    import concourse.tile as tile
    from contextlib import ExitStack

    f32 = mybir.dt.float32
    f32r = mybir.dt.float32r
    if KDT == "qk32":
        f16 = f32
    elif KDT == "f16":
        f16 = mybir.dt.float16
    else:
        f16 = mybir.dt.bfloat16
    bf16 = mybir.dt.bfloat16

    def rr(ap):
        # full-rate fp32 streaming for the fp32 Q/K chain; no-op for 16-bit
        return ap.bitcast(f32r) if KDT == "qk32" else ap
    EXP = mybir.ActivationFunctionType.Exp
    ADD = mybir.AluOpType.add
    MULT = mybir.AluOpType.mult

    nc = bacc.Bacc("TRN2", target_bir_lowering=False, debug=False, num_devices=8)

    xqT = nc.declare_dram_parameter("xqT", [D, S], f16, isOutput=False)
    xkT = nc.declare_dram_parameter("xkT", [D, S], f16, isOutput=False)
    xvT = nc.declare_dram_parameter("xvT", [D, S], bf16, isOutput=False)
    WqT = nc.declare_dram_parameter("WqT", [D, 512], f16, isOutput=False)
    WkT = nc.declare_dram_parameter("WkT", [D, 512], f16, isOutput=False)
    WvT = nc.declare_dram_parameter("WvT", [D, 512], bf16, isOutput=False)
    WoT = nc.declare_dram_parameter("WoT", [512, D], bf16, isOutput=False)
    bq_d = nc.declare_dram_parameter("bq", [512], f32, isOutput=False)
    bk_d = nc.declare_dram_parameter("bk", [512], f32, isOutput=False)
    bv_d = nc.declare_dram_parameter("bv", [1, 512], f32, isOutput=False)
    bo_d = nc.declare_dram_parameter("bo", [D], f32, isOutput=False)
    outT = nc.declare_dram_parameter("outT", [D, S], f32, isOutput=True)

    with ExitStack() as ctx:
        tc = ctx.enter_context(tile.TileContext(nc))
        persist = ctx.enter_context(tc.tile_pool(name="persist", bufs=1))
        xstage = ctx.enter_context(
            tc.tile_pool(name="xstage", bufs=2 if KDT == "qk32" else 3))
        ptp = ctx.enter_context(
            tc.tile_pool(name="ptp", bufs=6 if KDT == "qk32" else 8))
        ostage = ctx.enter_context(tc.tile_pool(name="ostage", bufs=2))
        astage = ctx.enter_context(tc.tile_pool(name="astage", bufs=2))
        nb = 2 if KDT == "qk32" else 4
        dnp = ctx.enter_context(tc.tile_pool(name="dnp", bufs=nb))
        bcp = ctx.enter_context(tc.tile_pool(name="bcp", bufs=nb))
        etp = ctx.enter_context(tc.tile_pool(name="etp", bufs=2, space="PSUM"))
        pvp = ctx.enter_context(tc.tile_pool(name="pvp", bufs=2, space="PSUM"))
        projp = ctx.enter_context(tc.tile_pool(name="projp", bufs=2, space="PSUM"))

        wq = persist.tile([P, NDT, 512], f16, tag="wq")
        wk = persist.tile([P, NDT, 512], f16, tag="wk")
        wv = persist.tile([P, NDT, 512], bf16, tag="wv")
        wo = persist.tile([P, 4, D], bf16, tag="wo")
        kT_sb = persist.tile([P, NHP, S], f16, tag="kT")
        qT_sb = persist.tile([P, NHP, S], f16, tag="qT")
        attT = persist.tile([P, NHP, S], bf16, tag="attT")
        v_pad = persist.tile([P, NKT, HL, DH + 1], bf16, tag="v_pad")
        bq_sb = persist.tile([P, 4], f32, tag="bq")
        bk_sb = persist.tile([P, 4], f32, tag="bk")
        bo_sb = persist.tile([P, NDT], f32, tag="bo")
        bv_bc = persist.tile([P, 512], f32, tag="bv_bc")

        # --- setup ---
        nc.sync.dma_start(rr(wq[:]), rr(WqT.rearrange("(k p) c -> p k c", p=P)))
        nc.sync.dma_start(rr(wk[:]), rr(WkT.rearrange("(k p) c -> p k c", p=P)))
        nc.sync.dma_start(wv[:], WvT.rearrange("(k p) c -> p k c", p=P))
        nc.sync.dma_start(wo[:], WoT.rearrange("(k p) c -> p k c", p=P))
        nc.sync.dma_start(bq_sb[:], bq_d.rearrange("(o p) -> p o", p=P))
        nc.sync.dma_start(bk_sb[:], bk_d.rearrange("(o p) -> p o", p=P))
        nc.sync.dma_start(bo_sb[:], bo_d.rearrange("(o p) -> p o", p=P))
        nc.sync.dma_start(bv_bc[:], bv_d[:].to_broadcast([P, 512]))
        nc.vector.memset(v_pad[:], 1.0)
        if BCAST == "pe":
            ones_sb = persist.tile([1, 64], f32, tag="ones")
            nc.vector.memset(ones_sb[:], 1.0)

        def load_x(x_dram, sc, name, dt=None):
            xt = xstage.tile([P, NDT, QC], dt or f16, tag="xstage", name=name)
            cast = (lambda a: a) if dt is not None else rr
            nc.sync.dma_start(
                cast(xt[:]),
                cast(x_dram[:, sc * QC:(sc + 1) * QC]
                     .rearrange("(k p) q -> p k q", p=P)),
            )
            return xt

        def kq_piece(w_tile, xt, bias_sb, out_sb, hps, sc):
            # projection in [dout, s] form: stationary W, moving x
            for hp in hps:
                ps = projp.tile([P, QC], f32, tag="projp")
                for kt in range(NDT):
                    nc.tensor.matmul(
                        out=ps[:],
                        lhsT=rr(w_tile[:, kt, hp * P:(hp + 1) * P]),
                        rhs=rr(xt[:, kt, :]),
                        start=(kt == 0), stop=(kt == NDT - 1),
                    )
                nc.vector.tensor_scalar_add(
                    rr(out_sb[:, hp, sc * QC:(sc + 1) * QC]), ps[:],
                    bias_sb[:, hp:hp + 1]
                )

        def v_piece(xt, st):
            # projection in [s, dout] form: stationary x, moving W
            so = (st % 4) * P
            ps = projp.tile([P, 512], f32, tag="projp")
            for kt in range(NDT):
                nc.tensor.matmul(
                    out=ps[:],
                    lhsT=xt[:, kt, so:so + P],
                    rhs=wv[:, kt, :],
                    start=(kt == 0), stop=(kt == NDT - 1),
                )
            nc.vector.tensor_tensor(
                v_pad[:, st, :, 0:DH],
                ps[:].rearrange("p (h d) -> p h d", h=HL),
                bv_bc[:].rearrange("p (h d) -> p h d", h=HL),
                ADD,
            )

        def attention_chunk(hp, qc):
            qsl = slice(qc * QC, (qc + 1) * QC)
            pv = [pvp.tile([P, QC], f32, tag="pvp", name=f"pv{hp}_{qc}_{h}")
                  for h in range(2)]
            for kt in range(NKT):
                et = etp.tile([P, 2, QC], f32, tag="etp")
                for h in range(2):
                    nc.tensor.matmul(
                        out=et[:, h, :],
                        lhsT=rr(kT_sb[h * 64:(h + 1) * 64, hp, kt * P:(kt + 1) * P]),
                        rhs=rr(qT_sb[h * 64:(h + 1) * 64, hp, qsl]),
                        start=True, stop=True,
                        tile_position=(h * 64, 0),
                    )
                pt_t = ptp.tile([P, 2, QC], bf16, tag="ptp")
                nc.scalar.activation(pt_t[:], et[:], EXP, scale=SCALE)
                for h in range(2):
                    nc.tensor.matmul(
                        out=pv[h][0:DH + 1, :],
                        lhsT=v_pad[:, kt, 2 * hp + h, 0:DH + 1],
                        rhs=pt_t[:, h, :],
                        start=(kt == 0), stop=(kt == NKT - 1),
                    )
            # normalize: denominator is PSUM row 64 (ones column of v_pad)
            dn = [dnp.tile([1, QC], f32, tag="dnp", name=f"dn{hp}_{qc}_{h}")
                  for h in range(2)]
            bc = [bcp.tile([64, QC], f32, tag="bcp", name=f"bc{hp}_{qc}_{h}")
                  for h in range(2)]
            for h in range(2):
                nc.vector.reciprocal(dn[h][:], pv[h][DH:DH + 1, :])
                if BCAST == "gpsimd":
                    nc.gpsimd.partition_broadcast(bc[h][:], dn[h][:], channels=64)
                else:
                    bp = etp.tile([P, 2, QC], f32, tag="etp", name=f"bcps{hp}_{qc}_{h}")
                    nc.tensor.matmul(out=bp[0:64, 0, :], lhsT=ones_sb[:],
                                     rhs=dn[h][:], start=True, stop=True)
                    nc.vector.tensor_copy(out=bc[h][:], in_=bp[0:64, 0, :])
            nc.vector.tensor_tensor(
                attT[0:64, hp, qsl], pv[0][0:64, :], bc[0][:], MULT
            )
            ast = astage.tile([64, QC], bf16, tag="astage")
            nc.vector.tensor_tensor(ast[:], pv[1][0:64, :], bc[1][:], MULT)
            nc.sync.dma_start(attT[64:128, hp, qsl], ast[:])

        def o_piece(sc):
            ssl = slice(sc * QC, (sc + 1) * QC)
            for dt in range(NDT):
                ps = projp.tile([P, QC], f32, tag="projp")
                for jt in range(4):
                    nc.tensor.matmul(
                        out=ps[:],
                        lhsT=wo[:, jt, dt * P:(dt + 1) * P],
                        rhs=attT[:, jt, ssl],
                        start=(jt == 0), stop=(jt == 3),
                    )
                ot = ostage.tile([P, QC], f32, tag="ostage")
                nc.vector.tensor_scalar_add(ot[:], ps[:], bo_sb[:, dt:dt + 1])
                nc.sync.dma_start(outT[dt * P:(dt + 1) * P, ssl], ot[:])

        # --- schedule ---
        for sc in range(4):
            kq_piece(wk, load_x(xkT, sc, f"xk{sc}"), bk_sb, kT_sb, [0], sc)
        for qc in range(4):
            kq_piece(wq, load_x(xqT, qc, f"xq{qc}"), bq_sb, qT_sb, [0], qc)
        for sc in range(4):
            xvt = load_x(xvT, sc, f"xv{sc}", dt=bf16)
            for sti in range(4):
                v_piece(xvt, sc * 4 + sti)
        for qc in range(NQC):
            attention_chunk(0, qc)
            kq_piece(wk, load_x(xkT, qc, f"xk2_{qc}"), bk_sb, kT_sb, [1, 2, 3], qc)
            kq_piece(wq, load_x(xqT, qc, f"xq2_{qc}"), bq_sb, qT_sb, [1, 2, 3], qc)
        for qc in range(NQC):
            attention_chunk(1, qc)
        for qc in range(NQC):
            attention_chunk(2, qc)
        for qc in range(NQC):
            attention_chunk(3, qc)
            o_piece(qc)

    if not nc.is_finalized():
        nc.finalize()
    return nc


def get_nc():
    if "nc" not in _CACHE:
        _CACHE["nc"] = _build_nc()
    return _CACHE["nc"]


def make_in_maps(inputs):
    import ml_dtypes

    bf16 = ml_dtypes.bfloat16
    if KDT == "qk32":
        f16 = np.float32
    elif KDT == "f16":
        f16 = np.float16
    else:
        f16 = ml_dtypes.bfloat16
    q = np.asarray(inputs["query"], np.float32)
    k = np.asarray(inputs["key"], np.float32)
    v = np.asarray(inputs["value"], np.float32)
    Wq = np.asarray(inputs["Wq"], np.float32)
    Wk = np.asarray(inputs["Wk"], np.float32)
    Wv = np.asarray(inputs["Wv"], np.float32)
    Wo = np.asarray(inputs["Wo"], np.float32)
    bq = np.asarray(inputs["bq"], np.float32)
    bk = np.asarray(inputs["bk"], np.float32)
    bv = np.asarray(inputs["bv"], np.float32)
    bo = np.asarray(inputs["bo"], np.float32)

    xT = {}
    for b in range(B):
        xT[b] = (
            np.ascontiguousarray(q[b].T).astype(f16),
            np.ascontiguousarray(k[b].T).astype(f16),
            np.ascontiguousarray(v[b].T).astype(bf16),
        )
    Wh = {}
    for hh in range(2):
        sl = slice(hh * 512, (hh + 1) * 512)
        Wh[hh] = {
            "WqT": np.ascontiguousarray(Wq.T[:, sl]).astype(f16),
            "WkT": np.ascontiguousarray(Wk.T[:, sl]).astype(f16),
            "WvT": np.ascontiguousarray(Wv.T[:, sl]).astype(bf16),
            "WoT": np.ascontiguousarray(Wo[:, sl].T).astype(bf16),
            "bq": np.ascontiguousarray(bq[sl]),
            "bk": np.ascontiguousarray(bk[sl]),
            "bv": np.ascontiguousarray(bv[sl]).reshape(1, 512),
            "bo": (bo * 0.5).astype(np.float32),
        }
    in_maps = []
    for c in range(8):
        b, hh = c // 2, c % 2
        m = dict(Wh[hh])
        m["xqT"], m["xkT"], m["xvT"] = xT[b]
        in_maps.append(m)
    return in_maps


def assemble(results):
    out = np.empty((B, S, D), np.float32)
    for b in range(B):
        acc = results[2 * b]["outT"].astype(np.float32) + \
            results[2 * b + 1]["outT"].astype(np.float32)
        out[b] = acc.T
    return out


def _numpy_fallback(inputs):
    q = np.asarray(inputs["query"], np.float64)
    k = np.asarray(inputs["key"], np.float64)
    v = np.asarray(inputs["value"], np.float64)
    Wq, bq = np.asarray(inputs["Wq"], np.float64), np.asarray(inputs["bq"], np.float64)
    Wk, bk = np.asarray(inputs["Wk"], np.float64), np.asarray(inputs["bk"], np.float64)
    Wv, bv = np.asarray(inputs["Wv"], np.float64), np.asarray(inputs["bv"], np.float64)
    Wo, bo = np.asarray(inputs["Wo"], np.float64), np.asarray(inputs["bo"], np.float64)
    qp = (q @ Wq.T + bq).reshape(B, S, H, DH).transpose(0, 2, 1, 3)
    kp = (k @ Wk.T + bk).reshape(B, S, H, DH).transpose(0, 2, 1, 3)
    vp = (v @ Wv.T + bv).reshape(B, S, H, DH).transpose(0, 2, 1, 3)
    e = np.einsum("bhqd,bhkd->bhqk", qp, kp) * SCALE
    mask = np.asarray(inputs["mask"])
    kpm = np.asarray(inputs["key_padding_mask"])
    e = np.where(mask == 0, -np.inf, e)
    e = np.where(kpm[:, None, None, :] == 0, -np.inf, e)
    e -= e.max(axis=-1, keepdims=True)
    p = np.exp(e)
    p /= p.sum(axis=-1, keepdims=True)
    o = np.einsum("bhqk,bhkd->bhqd", p, vp).transpose(0, 2, 1, 3).reshape(B, S, D)
    return (o @ Wo.T + bo).astype(np.float32)


def kernel(**inputs):
    mask = np.asarray(inputs["mask"])
    kpm = np.asarray(inputs["key_padding_mask"])
    if not (mask.all() and kpm.all()):
        return _numpy_fallback(inputs)
    from concourse.bass_utils import run_bass_kernel_spmd

    nc = get_nc()
    in_maps = make_in_maps(inputs)
    res = run_bass_kernel_spmd(nc, in_maps, list(range(8)))
    return assemble(res.results)
